# revision 1
# baseline (speedup 1.0000x reference)
"""Trainium2 Bass kernel: 16-head attention block (B=2, S=2048, H=1024).

Sharding: 8 cores = 2-way data parallel (batch) x 4-way tensor parallel
(head groups of 4 heads / 256 dims). Each core computes, for its batch
and head group:
    Q^T, K^T (= W @ x^T, [dims, seq] layout; Wq/bq pre-scaled by 1/8 on
    host so no score scaling is needed on device), V ([seq, dims]),
    S^T = K Q^T per head (key positions on partitions),
    P^T = exp(S^T + mask),
    ctx'^T = [V | 1]^T P^T    (ones column folded in -> row 64 = softmax
                               denominator),
    ctx^T normalized, then partial output O_g = ctx^T.T @ Wo[:,hs]^T.
Host sums the 4 partial outputs per batch and adds bo.

All matmuls run as float32r (full-rate fp32 mode on the PE array).
Emission order interleaves pair-1 projections under pair-0's ACT-bound
attention, and the output projection under pair-1's attention, so the
PE fills the softmax (scalar-engine) shadow.
"""

import contextlib

import numpy as np

import concourse.bass as bass
import concourse.mybir as mybir
import concourse.tile as tile
from concourse import bacc
from concourse.bass_utils import run_bass_kernel_spmd

B, S, H = 2, 2048, 1024
NUM_HEADS, HEAD_DIM = 16, 64
N_CORES = 8
GROUPS = 4                  # head-parallel groups per batch
HD = H // GROUPS            # 256 head-dims per core (4 heads)
P = 128
KT_H = H // P               # 8 k-tiles over hidden dim
KT_S = S // P               # 16 k-tiles over sequence (key positions)
NCH = 4                     # q chunks
CHUNK = S // NCH            # 512
F32 = mybir.dt.float32
F32R = mybir.dt.float32r
EXP = mybir.ActivationFunctionType.Exp

_PROGRAM_CACHE = {}


def _emit(tc, nc, dram, masked, with_bias):
    mm = nc.tensor.matmul
    xT_d, wq_d, wk_d, wv_d, wo_d, bq_d, bk_d, bv_d, am_d, o_d = dram

    stack = contextlib.ExitStack()
    with stack:
        const = stack.enter_context(tc.tile_pool(name="const", bufs=1))
        big = stack.enter_context(tc.tile_pool(name="big", bufs=1))

        onesf = const.tile([P, 64], F32)
        nc.any.memset(onesf[:], 1.0)
        ones64 = const.tile([P, 64], F32R)   # lane-64 row used as K=1 lhsT
        nc.vector.tensor_copy(ones64[:], onesf[:])
        # warm the ACT exp table before it is first needed
        trash = const.tile([1, 16], F32)
        nc.scalar.activation(trash[:], onesf[0:1, 0:16], EXP)
        if masked:
            amask_sb = const.tile([P, KT_S], F32)
            nc.sync.dma_start(out=amask_sb[:], in_=am_d[:])
        if with_bias:
            ones_sb = const.tile([1, 512], F32R)
            for i in range(8):
                nc.vector.tensor_copy(ones_sb[0:1, i * 64:(i + 1) * 64],
                                      onesf[0:1, :])
            bq_sb = const.tile([1, HD], F32R)
            nc.sync.dma_start(out=bq_sb[:], in_=bq_d[:])
            bk_sb = const.tile([1, HD], F32R)
            nc.sync.dma_start(out=bk_sb[:], in_=bk_d[:])
            bv_sb = const.tile([1, HD], F32R)
            nc.sync.dma_start(out=bv_sb[:], in_=bv_d[:])
        wo_sb = const.tile([P, HD // P, H], F32R)

        # persistent activations
        qT_sb = big.tile([P, 2, S], F32R)    # [dim-in-pair, pair, seq]
        kT_sb = big.tile([P, 2, S], F32R)
        v_sb = big.tile([P, KT_S, GROUPS, HEAD_DIM + 1], F32R)  # [seq, kt, head, d+1]
        ctxT_sb = big.tile([P, 2, S], F32R)

        # ones column of V' (the rowsum trick)
        nc.vector.tensor_copy(v_sb[:, :, :, HEAD_DIM:HEAD_DIM + 1], onesf[:, 0:KT_S * GROUPS])

        # ---------- input DMAs (wk/xT first so compute starts early) ----------
        kq_pool = tc.alloc_tile_pool(name="kq_pool", bufs=1, side="right")
        wv_stack = contextlib.ExitStack()
        wv_pool = wv_stack.enter_context(tc.tile_pool(name="wv_pool", bufs=1, side="right"))
        wk_sb = kq_pool.tile([P, KT_H, HD], F32R)
        xT_sb = kq_pool.tile([P, KT_H, S], F32R)
        wq_sb = kq_pool.tile([P, KT_H, HD], F32R)
        wv_sb = wv_pool.tile([P, KT_H, HD], F32R)
        for kt in range(KT_H):
            nc.sync.dma_start(out=wk_sb[:, kt, :], in_=wk_d[kt * P:(kt + 1) * P, :])
        for kt in range(KT_H):
            nc.sync.dma_start(out=wq_sb[:, kt, :], in_=wq_d[kt * P:(kt + 1) * P, :])
        # x^T arrives column-chunked: seq-chunk cc unlocks K/Q projections,
        # V tiles, and score tiles for that quarter of the sequence, so the
        # attention stream starts ~3MB into the 13MB inbound DMA.
        for cc in range(4):
            for kt in range(KT_H):
                nc.sync.dma_start(
                    out=xT_sb[:, kt, cc * 512:(cc + 1) * 512],
                    in_=xT_d[kt * P:(kt + 1) * P, cc * 512:(cc + 1) * 512])
        for kt in range(KT_H):
            nc.sync.dma_start(out=wv_sb[:, kt, :], in_=wv_d[kt * P:(kt + 1) * P, :])

        # all projection psum traffic lives in two persistent banks (tags
        # ps_k / ps_q, alternating for double-buffering) so the attention
        # pool can hold its six banks for the whole kernel with no
        # pool-boundary barrier between projections and attention
        def qk_proj(ps_pool, w_sb, b_sb, dst, p):
            for c4 in range(4):
                ps_qk = ps_pool.tile([P, 512], F32, bufs=1, name="ps_qk",
                                     tag="ps_k" if c4 % 2 == 0 else "ps_q")
                for kt in range(KT_H):
                    mm(ps_qk[:],
                       w_sb[:, kt, p * P:(p + 1) * P],
                       xT_sb[:, kt, c4 * 512:(c4 + 1) * 512],
                       start=(kt == 0), stop=(not with_bias and kt == KT_H - 1))
                if with_bias:
                    mm(ps_qk[:],
                       b_sb[:, p * P:(p + 1) * P],
                       ones_sb[:, 0:512],
                       start=False, stop=True)
                nc.vector.tensor_copy(dst[:, p, c4 * 512:(c4 + 1) * 512], ps_qk[:])

        def oproj_quarter(o_ps, opool, q):
            for m in range(4 * q, 4 * q + 4):
                o_sb = opool.tile([P, H], F32, tag="o_sb", bufs=3)
                for n2 in range(2):
                    ps_o = o_ps.tile([P, 512], F32, tag="ps_o", bufs=2)
                    for kt2 in range(HD // P):
                        mm(ps_o[:],
                           ctxT_sb[:, kt2, m * P:(m + 1) * P],
                           wo_sb[:, kt2, n2 * 512:(n2 + 1) * 512],
                           start=(kt2 == 0), stop=(kt2 == HD // P - 1))
                    nc.vector.tensor_copy(o_sb[:, n2 * 512:(n2 + 1) * 512], ps_o[:])
                nc.sync.dma_start(out=o_d[m * P:(m + 1) * P, :], in_=o_sb[:])

        def v_proj(ms, pool):
            for m in ms:
                ps_v = pool.tile([P, HD], F32, bufs=1, name="ps_v",
                                 tag="ps_k" if m % 2 == 0 else "ps_q")
                for kt in range(KT_H):
                    mm(ps_v[:],
                       xT_sb[:, kt, m * P:(m + 1) * P],
                       wv_sb[:, kt, :],
                       start=(kt == 0), stop=(not with_bias and kt == KT_H - 1))
                if with_bias:
                    mm(ps_v[:],
                       ones_sb[:, 0:P],
                       bv_sb[:],
                       start=False, stop=True)
                nc.vector.tensor_copy(v_sb[:, m, :, 0:HEAD_DIM], ps_v[:])

        # ---------- pools: attention psum first (banks 0-5, alive for the
        # whole kernel), projections in the remaining two banks ----------
        attn_stack = contextlib.ExitStack()
        a_ps = attn_stack.enter_context(
            tc.tile_pool(name="attn_psum", bufs=1, space="PSUM"))
        ptp = attn_stack.enter_context(
            tc.tile_pool(name="pt_pool", bufs=4 if not (masked or with_bias) else 2))
        npool = attn_stack.enter_context(tc.tile_pool(name="norm_pool", bufs=2))
        psA = tc.alloc_tile_pool(name="proj_psum", bufs=1, space="PSUM")

        # pair-0 K/Q projections, seq-chunk-outer so each inbound x
        # column-chunk is consumed as soon as it lands; score tiles for
        # seq-chunk 0 can then start ~3MB into the inbound DMA
        for cc in range(4):
            ps_k = psA.tile([P, 512], F32, tag="ps_k", bufs=1, name="ps_k")
            ps_q = psA.tile([P, 512], F32, tag="ps_q", bufs=1, name="ps_q")
            for kt in range(KT_H):
                for ps, w_sb in ((ps_k, wk_sb), (ps_q, wq_sb)):
                    mm(ps[:],
                       w_sb[:, kt, 0:P],
                       xT_sb[:, kt, cc * 512:(cc + 1) * 512],
                       start=(kt == 0),
                       stop=(not with_bias and kt == KT_H - 1))
            if with_bias:
                for ps, b_sb in ((ps_k, bk_sb), (ps_q, bq_sb)):
                    mm(ps[:],
                       b_sb[:, 0:P],
                       ones_sb[:, 0:512],
                       start=False, stop=True)
            nc.vector.tensor_copy(kT_sb[:, 0, cc * 512:(cc + 1) * 512], ps_k[:])
            nc.vector.tensor_copy(qT_sb[:, 0, cc * 512:(cc + 1) * 512], ps_q[:])

        v_proj(range(KT_S), psA)

        # pair-0 attention; V projection + pair-1 projections fill the PE
        # shadow under the ACT-bound softmax
        for c in range(NCH):
            _attn_one_chunk(tc, nc, a_ps, ptp, npool, 0, c, masked,
                            amask_sb if masked else None,
                            kT_sb, qT_sb, v_sb, ctxT_sb, ones64)
        qk_proj(psA, wk_sb, bk_sb if with_bias else None, kT_sb, 1)
        qk_proj(psA, wq_sb, bq_sb if with_bias else None, qT_sb, 1)
        wv_stack.close()

        # wo arrives during pair-0 attention; needed only in the final phase
        nc.sync.dma_start(out=wo_sb[:], in_=wo_d.rearrange("(t p) c -> p t c", p=P))
        kq_pool.release()
        psA.release()

        # pair-1 attention with the output projection interleaved per chunk
        o_ps = attn_stack.enter_context(tc.tile_pool(name="o_psum", bufs=1, space="PSUM"))
        opool = attn_stack.enter_context(tc.tile_pool(name="o_pool", bufs=1))
        for c in range(NCH):
            _attn_one_chunk(tc, nc, a_ps, ptp, npool, 1, c, masked,
                            amask_sb if masked else None,
                            kT_sb, qT_sb, v_sb, ctxT_sb, ones64)
            oproj_quarter(o_ps, opool, c)
        attn_stack.close()


def _attn_one_chunk(tc, nc, psum, ptp, npool, p, c, masked, amask_sb,
                    kT_sb, qT_sb, v_sb, ctxT_sb, ones64):
    mm = nc.tensor.matmul
    ctx_e = psum.tile([HEAD_DIM + 1, CHUNK], F32, tag="ctx_e", bufs=1)
    ctx_o = psum.tile([HEAD_DIM + 1, CHUNK], F32, tag="ctx_o", bufs=1)
    for kt in range(KT_S):
        s_pair = psum.tile([P, 2 * CHUNK], F32, tag="s_pair", bufs=2)
        for hl in range(2):
            mm(s_pair[:, hl * CHUNK:(hl + 1) * CHUNK],
               kT_sb[hl * 64:(hl + 1) * 64, p, kt * P:(kt + 1) * P],
               qT_sb[hl * 64:(hl + 1) * 64, p, c * CHUNK:(c + 1) * CHUNK],
               start=True, stop=True)
        pt = ptp.tile([P, 2 * CHUNK], F32R, tag="pt")
        if masked:
            nc.scalar.activation(pt[:], s_pair[:], EXP, bias=amask_sb[:, kt:kt + 1])
        else:
            nc.scalar.activation(pt[:], s_pair[:], EXP)
        for hl in range(2):
            ctx = ctx_e if hl == 0 else ctx_o
            mm(ctx[:],
               v_sb[:, kt, 2 * p + hl, :],
               pt[:, hl * CHUNK:(hl + 1) * CHUNK],
               start=(kt == 0), stop=(kt == KT_S - 1))
    ctxu = npool.tile([HEAD_DIM + 1, 2, CHUNK], F32, tag="ctxu", bufs=2)
    nc.vector.tensor_copy(ctxu[:, 0, :], ctx_e[:])
    nc.vector.tensor_copy(ctxu[:, 1, :], ctx_o[:])
    recip_sb = npool.tile([HEAD_DIM + 1, 2, CHUNK], F32R, tag="recip", bufs=2)
    with nc.allow_low_precision(reason="softmax denominators are O(1e3); fp32r's 11-bit mantissa is plenty"):
        nc.vector.reciprocal(recip_sb[64:65, 0, :], ctxu[64:65, 0, :])
        nc.vector.reciprocal(recip_sb[64:65, 1, :], ctxu[64:65, 1, :])
    bc_e = psum.tile([HEAD_DIM, CHUNK], F32, tag="ctx_e", bufs=1)
    bc_o = psum.tile([HEAD_DIM, CHUNK], F32, tag="ctx_o", bufs=1)
    for hl in range(2):
        mm(bc_e if hl == 0 else bc_o,
           ones64[64:65, :],
           recip_sb[64:65, hl, :],
           start=True, stop=True)
    nc.vector.tensor_mul(ctxT_sb[0:64, p, c * CHUNK:(c + 1) * CHUNK],
                         ctxu[0:64, 0, :], bc_e[:])
    tmp_o = npool.tile([HEAD_DIM, CHUNK], F32R, tag="tmp_o", bufs=2)
    nc.vector.tensor_mul(tmp_o[:], ctxu[0:64, 1, :], bc_o[:])
    nc.sync.dma_start(out=ctxT_sb[64:128, p, c * CHUNK:(c + 1) * CHUNK],
                      in_=tmp_o[:])


def build_program(masked=False, with_bias=False):
    key = (masked, with_bias)
    if key in _PROGRAM_CACHE:
        return _PROGRAM_CACHE[key]
    nc = bacc.Bacc("TRN2", target_bir_lowering=False, debug=False,
                   enable_asserts=False)
    xT = nc.dram_tensor("xT", [H, S], F32R, kind="ExternalInput").ap()
    wq = nc.dram_tensor("wq", [H, HD], F32R, kind="ExternalInput").ap()
    wk = nc.dram_tensor("wk", [H, HD], F32R, kind="ExternalInput").ap()
    wv = nc.dram_tensor("wv", [H, HD], F32R, kind="ExternalInput").ap()
    wo = nc.dram_tensor("wo", [HD, H], F32R, kind="ExternalInput").ap()
    bq = nc.dram_tensor("bq", [1, HD], F32R, kind="ExternalInput").ap()
    bk = nc.dram_tensor("bk", [1, HD], F32R, kind="ExternalInput").ap()
    bv = nc.dram_tensor("bv", [1, HD], F32R, kind="ExternalInput").ap()
    am = nc.dram_tensor("am", [P, KT_S], F32, kind="ExternalInput").ap()
    o = nc.dram_tensor("o_part", [S, H], F32, kind="ExternalOutput").ap()
    with tile.TileContext(nc) as tc:
        _emit(tc, nc, (xT, wq, wk, wv, wo, bq, bk, bv, am, o), masked, with_bias)
    nc.compile()
    _PROGRAM_CACHE[key] = nc
    return nc


def _round_fp32r(a):
    """Round fp32 to the PE's fp32r format (11 mantissa bits, RNE)."""
    u = np.ascontiguousarray(a, np.float32).view(np.uint32)
    r = (u + np.uint32(0x7FF) + ((u >> np.uint32(12)) & np.uint32(1))) \
        & np.uint32(0xFFFFF000)
    return r.view(np.float32)


def make_in_maps(hidden_states, attention_mask, Wq, bq, Wk, bk, Wv, bv, Wo, bo):
    """Per-core input dicts. Core c: batch c//4, head-group c%4.

    Wq/bq are pre-scaled by 1/8 (= 1/sqrt(HEAD_DIM), exact in fp32) so the
    kernel's raw scores are already scaled. Tensors feeding float32r
    matmuls are pre-rounded to fp32r on the host (the device DMAs them
    into float32r tiles verbatim).
    """
    hidden_states = np.asarray(hidden_states, np.float32)
    attention_mask = np.asarray(attention_mask, np.float32)
    xTs = [_round_fp32r(hidden_states[b].T) for b in range(B)]
    ams = []
    for b in range(B):
        amask = ((1.0 - attention_mask[b]) * -10000.0).astype(np.float32)
        ams.append(np.ascontiguousarray(amask.reshape(KT_S, P).T))
    in_maps = []
    for c in range(N_CORES):
        b, g = divmod(c, GROUPS)
        hs = slice(g * HD, (g + 1) * HD)
        in_maps.append({
            "xT": xTs[b],
            "wq": _round_fp32r(np.asarray(Wq, np.float32)[hs, :].T * np.float32(0.125)),
            "wk": _round_fp32r(np.asarray(Wk, np.float32)[hs, :].T),
            "wv": _round_fp32r(np.asarray(Wv, np.float32)[hs, :].T),
            "wo": _round_fp32r(np.asarray(Wo, np.float32)[:, hs].T),
            "bq": _round_fp32r(np.asarray(bq, np.float32)[hs].reshape(1, HD) * np.float32(0.125)),
            "bk": _round_fp32r(np.asarray(bk, np.float32)[hs].reshape(1, HD)),
            "bv": _round_fp32r(np.asarray(bv, np.float32)[hs].reshape(1, HD)),
            "am": ams[b],
        })
    return in_maps


def kernel(hidden_states, attention_mask, Wq, bq, Wk, bk, Wv, bv, Wo, bo):
    masked = not bool(np.all(np.asarray(attention_mask) == 1.0))
    with_bias = not (np.all(np.asarray(bq) == 0) and np.all(np.asarray(bk) == 0)
                     and np.all(np.asarray(bv) == 0))
    nc = build_program(masked, with_bias)
    in_maps = make_in_maps(hidden_states, attention_mask,
                           Wq, bq, Wk, bk, Wv, bv, Wo, bo)
    res = run_bass_kernel_spmd(nc, in_maps, core_ids=list(range(N_CORES)))
    out = np.zeros((B, S, H), np.float32)
    for c in range(N_CORES):
        b = c // GROUPS
        out[b] += res.results[c]["o_part"]
    out += np.asarray(bo, np.float32)
    return out



# revision 7
# speedup vs baseline: 1.1963x; 1.1963x over previous
"""Trainium2 Bass kernel: 16-head attention block (B=2, S=2048, H=1024).

Sharding: 8 cores = 2-way data parallel (batch) x 4-way tensor parallel
(head groups of 4 heads / 256 dims). Each core computes, for its batch
and head group:
    Q^T, K^T (= W @ x^T, [dims, seq] layout; Wq/bq pre-scaled by 1/8 on
    host so no score scaling is needed on device), V ([seq, dims]),
    S^T = K Q^T per head (key positions on partitions),
    P^T = exp(S^T + mask)  (bf16),
    ctx  = P^T.T @ [V | 1 | 0] per 128-query block (full 128-wide
           contraction; col 64 = softmax denominator, col 65 = pad),
    ctx normalized on DVE, DMA-transposed to ctx^T, then partial output
    O_g = ctx^T.T @ Wo[:,hs]^T.
Host sums the 4 partial outputs per batch and adds bo.

Projections and scores run in float32r (full-rate on the PE at free
size >= 256).  The probability matrix and V are bf16 so the flipped ctx
matmuls (N=66) keep LDWEIGHTS (fast-weight-load) under the 66-cycle
streams.  The whole kernel is one software-pipelined loop over 128
(pair, chunk, kt) slots: each slot emits the NEXT slot's score matmuls,
then a small PE "filler" piece (pair-1 projections / output projection),
then this slot's ctx matmuls, so the in-order PE queue never parks
behind an exp wait and the ACT engine (the 1.04us/slot bottleneck)
stays saturated.
"""

import contextlib
from collections import defaultdict

import numpy as np

import concourse.bass as bass
import concourse.mybir as mybir
import concourse.tile as tile
from concourse import bacc
from concourse.bass_utils import run_bass_kernel_spmd
from concourse.masks import make_identity

B, S, H = 2, 2048, 1024
NUM_HEADS, HEAD_DIM = 16, 64
N_CORES = 8
GROUPS = 4                  # head-parallel groups per batch
HD = H // GROUPS            # 256 head-dims per core (4 heads)
P = 128
KT_H = H // P               # 8 k-tiles over hidden dim
KT_S = S // P               # 16 k-tiles over sequence (key positions)
NCH = 4                     # q chunks per head pair
CHUNK = S // NCH            # 512
NQB = CHUNK // P            # 4 query blocks per chunk
NV = 66                     # V row: 64 dims + ones col + pad col
F32 = mybir.dt.float32
F32R = mybir.dt.float32r
BF16 = mybir.dt.bfloat16
EXP = mybir.ActivationFunctionType.Exp

_PROGRAM_CACHE = {}


def _emit(tc, nc, dram, masked, with_bias):
    mm = nc.tensor.matmul
    xT_d, wq_d, wk_d, wv_d, wo_d, bq_d, bk_d, bv_d, am_d, o_d = dram

    stack = contextlib.ExitStack()
    with stack:
        const = stack.enter_context(tc.tile_pool(name="const", bufs=1))
        big = stack.enter_context(tc.tile_pool(name="big", bufs=1))

        onesf = const.tile([P, 64], F32)
        nc.any.memset(onesf[:], 1.0)
        # warm the ACT exp table before it is first needed
        trash = const.tile([1, 16], F32)
        nc.scalar.activation(trash[:], onesf[0:1, 0:16], EXP)
        if masked:
            amask_sb = const.tile([P, KT_S], F32)
            nc.sync.dma_start(out=amask_sb[:], in_=am_d[:])
        if with_bias:
            ones_sb = const.tile([1, 512], F32R)
            for i in range(8):
                nc.vector.tensor_copy(ones_sb[0:1, i * 64:(i + 1) * 64],
                                      onesf[0:1, :])
            bq_sb = const.tile([1, HD], F32R)
            nc.sync.dma_start(out=bq_sb[:], in_=bq_d[:])
            bk_sb = const.tile([1, HD], F32R)
            nc.sync.dma_start(out=bk_sb[:], in_=bk_d[:])
            bv_sb = const.tile([1, HD], F32R)
            nc.sync.dma_start(out=bv_sb[:], in_=bv_d[:])
        wo_sb = const.tile([P, HD // P, H], BF16)

        # persistent activations
        qT_sb = big.tile([P, 2, S], F32R)    # [dim-in-pair, pair, seq]
        kT_sb = big.tile([P, 2, S], F32R)
        v_sb = big.tile([P, KT_S, GROUPS, NV], BF16)  # [seq, kt, head, 66]
        ctxT_sb = big.tile([P, 2, S], BF16)

        # ones column (softmax denominator) and zero pad column of V'
        nc.any.memset(v_sb[:, :, :, HEAD_DIM:HEAD_DIM + 1], 1.0)
        nc.any.memset(v_sb[:, :, :, HEAD_DIM + 1:NV], 0.0)

        # identity for the tail's PE transposes
        ident = const.tile([P, P], BF16)
        make_identity(nc, ident[:])
        warm_src = const.tile([P, 512], BF16)
        nc.any.memset(warm_src[:], 0.0)

        # ---------- input DMAs: wk, wq, xT-cc0 gate the first exp; wv gates
        # chunk-0 ctx; later xT chunks stream under chunk-0's slots ----------
        kq_pool = tc.alloc_tile_pool(name="kq_pool", bufs=1, side="right")
        wv_stack = contextlib.ExitStack()
        wv_pool = wv_stack.enter_context(tc.tile_pool(name="wv_pool", bufs=1, side="right"))
        wk_sb = kq_pool.tile([P, KT_H, HD], F32R)
        xT_sb = kq_pool.tile([P, KT_H, S], F32R)
        wq_sb = kq_pool.tile([P, KT_H, HD], F32R)
        wv_sb = wv_pool.tile([P, KT_H, HD], F32R)
        xT_dr = xT_d.rearrange("(t p) s -> p t s", p=P)
        nc.sync.dma_start(out=wk_sb[:], in_=wk_d.rearrange("(t p) c -> p t c", p=P))
        nc.sync.dma_start(out=wq_sb[:], in_=wq_d.rearrange("(t p) c -> p t c", p=P))

        def xT_cc_dma(cc):
            for h in range(2):
                nc.sync.dma_start(
                    out=xT_sb[:, 4 * h:4 * h + 4, cc * 512:(cc + 1) * 512],
                    in_=xT_dr[:, 4 * h:4 * h + 4, cc * 512:(cc + 1) * 512])
        xT_cc_dma(0)
        nc.sync.dma_start(out=wv_sb[:], in_=wv_d.rearrange("(t p) c -> p t c", p=P))
        for cc in range(1, 4):
            xT_cc_dma(cc)
        nc.sync.dma_start(out=wo_sb[:], in_=wo_d.rearrange("(t p) c -> p t c", p=P))

        # ---------- PSUM pools: 4 banks scores (double-buffered), 2 banks
        # ctx accumulators, 2 banks misc (projections / output proj) ----------
        attn_stack = contextlib.ExitStack()
        sp_pool = attn_stack.enter_context(
            tc.tile_pool(name="sp_psum", bufs=2, space="PSUM"))
        acc_pool = attn_stack.enter_context(
            tc.tile_pool(name="acc_psum", bufs=1, space="PSUM"))
        misc_ps = attn_stack.enter_context(
            tc.tile_pool(name="misc_psum", bufs=2, space="PSUM"))
        ptp = attn_stack.enter_context(tc.tile_pool(name="pt_pool", bufs=3))
        npool = attn_stack.enter_context(tc.tile_pool(name="norm_pool", bufs=2))
        opool = attn_stack.enter_context(tc.tile_pool(name="o_pool", bufs=2))

        # PE p-state warm-up: burn the cold/mid HAM states on dummy matmuls
        # while the first input DMAs are still in flight.
        warm_ps = misc_ps.tile([P, 512], F32, tag="misc", name="warm_ps")
        for _ in range(20):
            mm(warm_ps[:], warm_src[:, 0:P], warm_src[:], start=True, stop=True)

        # ---- filler piece builders: each returned callable is <= ~0.9us PE ----
        def qk_proj_pieces(w_sb, b_sb, dst, p, cc, nsplit=2):
            """Split one K/Q projection group into `nsplit` pieces."""
            state = {}
            bounds = [KT_H * i // nsplit for i in range(nsplit + 1)]
            def piece(i):
                def run():
                    if i == 0:
                        state["ps"] = misc_ps.tile([P, 512], F32, tag="misc", name="ps_qk")
                    ps = state["ps"]
                    for kt in range(bounds[i], bounds[i + 1]):
                        mm(ps[:],
                           w_sb[:, kt, p * P:(p + 1) * P],
                           xT_sb[:, kt, cc * 512:(cc + 1) * 512],
                           start=(kt == 0),
                           stop=(not with_bias and kt == KT_H - 1))
                    if i == nsplit - 1:
                        if with_bias:
                            mm(ps[:], b_sb[:, p * P:(p + 1) * P],
                               ones_sb[:, 0:512], start=False, stop=True)
                        nc.vector.tensor_copy(
                            dst[:, p, cc * 512:(cc + 1) * 512], ps[:])
                return run
            return [piece(i) for i in range(nsplit)]

        def v_proj_piece(m):
            def run():
                ps = misc_ps.tile([P, 512], F32, tag="misc", name="ps_v")
                for kt in range(KT_H):
                    mm(ps[:, 0:HD],
                       xT_sb[:, kt, m * P:(m + 1) * P],
                       wv_sb[:, kt, :],
                       start=(kt == 0), stop=(not with_bias and kt == KT_H - 1))
                if with_bias:
                    mm(ps[:, 0:HD], ones_sb[:, 0:P], bv_sb[:],
                       start=False, stop=True)
                nc.vector.tensor_copy(
                    v_sb[:, m, :, 0:HEAD_DIM],
                    ps[:, 0:HD].rearrange("s (h d) -> s h d", d=HEAD_DIM))
            return run

        def oproj_pieces(m, on_act=False):
            state = {}
            def pe_piece(n2):
                def run():
                    ps = misc_ps.tile([P, 512], F32, tag="misc", name="ps_o")
                    state[n2] = ps
                    for kt2 in range(HD // P):
                        mm(ps[:],
                           ctxT_sb[:, kt2, m * P:(m + 1) * P],
                           wo_sb[:, kt2, n2 * 512:(n2 + 1) * 512],
                           start=(kt2 == 0), stop=(kt2 == HD // P - 1))
                return run
            def out_piece():
                o_sb = opool.tile([P, H], BF16, tag="o_sb", name="o_sb")
                for n2 in range(2):
                    if on_act:
                        nc.scalar.copy(o_sb[:, n2 * 512:(n2 + 1) * 512],
                                       state[n2][:])
                    else:
                        nc.vector.tensor_copy(
                            o_sb[:, n2 * 512:(n2 + 1) * 512], state[n2][:])
                nc.sync.dma_start(out=o_d[m * P:(m + 1) * P, :], in_=o_sb[:])
            return [pe_piece(0), pe_piece(1), out_piece]

        # ---------------- attention slot machinery ----------------
        def scores_kt(p, c, kt):
            """S^T tile for both heads of pair p: [128 keys, 2*512 q]."""
            sp = sp_pool.tile([P, 2, CHUNK], F32, tag="s_pair", name="sp")
            for hl in range(2):
                mm(sp[:, hl, :],
                   kT_sb[hl * 64:(hl + 1) * 64, p, kt * P:(kt + 1) * P],
                   qT_sb[hl * 64:(hl + 1) * 64, p, c * CHUNK:(c + 1) * CHUNK],
                   start=True, stop=True)
            pt = ptp.tile([P, 2, CHUNK], BF16, tag="pt", name="pt")
            if masked:
                nc.scalar.activation(pt[:], sp[:], EXP,
                                     bias=amask_sb[:, kt:kt + 1])
            else:
                nc.scalar.activation(pt[:], sp[:], EXP)
            return pt

        def ctx_kt(p, c, kt, acc, pt):
            """Accumulate ctx blocks [128 q, 66] for all (hl, qb)."""
            for hl in range(2):
                for qb in range(NQB):
                    mm(acc[:, hl, qb * NV:(qb + 1) * NV],
                       pt[:, hl, qb * P:(qb + 1) * P],
                       v_sb[:, kt, 2 * p + hl, :],
                       start=(kt == 0 and qb == 0),
                       stop=(kt == KT_S - 1 and qb == NQB - 1))

        def finish_chunk(p, c, acc):
            """Normalize ctx (DVE) and DMA-transpose into ctxT_sb."""
            rec = npool.tile([P, 2, NQB, 1], F32, tag="rec", name="rec")
            ctx_n = npool.tile([P, NQB, 2, HEAD_DIM], BF16, tag="ctx_n", name="ctx_n")
            for hl in range(2):
                blk = acc[:, hl, 0:NQB * NV].rearrange(
                    "q (qb c) -> q qb c", c=NV)
                nc.vector.reciprocal(rec[:, hl], blk[:, :, 64:65])
                nc.vector.tensor_mul(
                    ctx_n[:, :, hl, :], blk[:, :, 0:HEAD_DIM],
                    rec[:, hl].broadcast_to((P, NQB, HEAD_DIM)))
            for qb in range(NQB):
                nc.sync.dma_start_transpose(
                    out=ctxT_sb[:, p, c * CHUNK + qb * P:c * CHUNK + (qb + 1) * P],
                    in_=ctx_n[:, qb])

        def finish_chunk_tail(p, c, acc):
            """Last chunk: per-qblock normalize + PE transpose + output
            projection, pipelined so the tail is as short as possible."""
            rec = npool.tile([P, 2, NQB, 1], F32, tag="rec", name="rec")
            ctx_n = npool.tile([P, NQB, 2, HEAD_DIM], BF16, tag="ctx_n", name="ctx_n")
            blks = [acc[:, hl, 0:NQB * NV].rearrange("q (qb c) -> q qb c", c=NV)
                    for hl in range(2)]
            for qb in range(NQB):
                for hl in range(2):
                    nc.vector.reciprocal(rec[:, hl, qb], blks[hl][:, qb, 64:65])
                    nc.vector.tensor_mul(
                        ctx_n[:, qb, hl, :], blks[hl][:, qb, 0:HEAD_DIM],
                        rec[:, hl, qb].broadcast_to((P, HEAD_DIM)))
                tr_ps = misc_ps.tile([P, 512], F32, tag="misc", name="tr_ps")
                tr_bf = tr_ps[:].bitcast(BF16)
                nc.tensor.transpose(tr_bf[:, 0:P], ctx_n[:, qb], ident[:])
                dst = ctxT_sb[:, p, c * CHUNK + qb * P:c * CHUNK + (qb + 1) * P]
                nc.vector.tensor_copy(dst, tr_bf[:, 0:P])
                m = c * NQB + qb
                for pc in oproj_pieces(m, on_act=True):
                    pc()

        # ---------------- piece schedule over the 128 slots ----------------
        # slot index = (p*NCH + c)*KT_S + kt
        sched = defaultdict(list)
        # chunk 0 (slots 0-15): V projection for tile m in slot m; pair-0
        # K/Q for column-chunk cc land in slots 4cc-4 / 4cc-3 (ready before
        # the slot-(4cc-1) emission of scores kt=4cc).
        for m in range(KT_S):
            sched[m].append(v_proj_piece(m))
        for cc in range(1, 4):
            for i, pc in enumerate(qk_proj_pieces(wk_sb,
                                                  bk_sb if with_bias else None,
                                                  kT_sb, 0, cc)):
                sched[4 * (cc - 1)].append(pc)
            for i, pc in enumerate(qk_proj_pieces(wq_sb,
                                                  bq_sb if with_bias else None,
                                                  qT_sb, 0, cc)):
                sched[4 * (cc - 1) + 1].append(pc)
        # pair-1 K/Q projections: 8 groups split into 4 pieces each, spread
        # over the slots of pair-0 chunks 1..2 (two pieces per slot).
        p1_pieces = []
        for cc in range(4):
            p1_pieces += qk_proj_pieces(wk_sb, bk_sb if with_bias else None,
                                        kT_sb, 1, cc, nsplit=4)
            p1_pieces += qk_proj_pieces(wq_sb, bq_sb if with_bias else None,
                                        qT_sb, 1, cc, nsplit=4)
        for i, pc in enumerate(p1_pieces):
            sched[16 + i].append(pc)
        # output projection for chunk c of the full ctx: slots of pair-1
        # chunk c+1 (chunk 3's pieces run after the loop).
        for c in range(NCH - 1):
            for m in range(4 * c, 4 * c + 4):
                base = (NCH + c + 1) * KT_S
                for i, pc in enumerate(oproj_pieces(m)):
                    sched[base + (m % 4) * 4 + i].append(pc)

        # ---------------- phase 1: pair-0 cc0 K/Q projection ----------------
        ps_k = misc_ps.tile([P, 512], F32, tag="misc")
        for kt in range(KT_H):
            mm(ps_k[:], wk_sb[:, kt, 0:P], xT_sb[:, kt, 0:512],
               start=(kt == 0), stop=(not with_bias and kt == KT_H - 1))
        if with_bias:
            mm(ps_k[:], bk_sb[:, 0:P], ones_sb[:, 0:512], start=False, stop=True)
        nc.vector.tensor_copy(kT_sb[:, 0, 0:512], ps_k[:])
        ps_q = misc_ps.tile([P, 512], F32, tag="misc")
        for kt in range(KT_H):
            mm(ps_q[:], wq_sb[:, kt, 0:P], xT_sb[:, kt, 0:512],
               start=(kt == 0), stop=(not with_bias and kt == KT_H - 1))
        if with_bias:
            mm(ps_q[:], bq_sb[:, 0:P], ones_sb[:, 0:512], start=False, stop=True)
        nc.vector.tensor_copy(qT_sb[:, 0, 0:512], ps_q[:])

        # ---------------- phase 2: the software-pipelined slot loop ----------
        slots = [(p, c, kt)
                 for p in range(2) for c in range(NCH) for kt in range(KT_S)]
        acc = None
        pt_next = scores_kt(0, 0, 0)
        for i, (p, c, kt) in enumerate(slots):
            pt_cur = pt_next
            if i + 1 < len(slots):
                pn, cn, ktn = slots[i + 1]
                pt_next = scores_kt(pn, cn, ktn)
            for piece in sched.get(i, ()):
                piece()
            if kt == 0:
                acc = acc_pool.tile([P, 2, 512], F32, tag="acc", name="acc")
            ctx_kt(p, c, kt, acc, pt_cur)
            if kt == KT_S - 1:
                if (p, c) == (1, NCH - 1):
                    finish_chunk_tail(p, c, acc)
                else:
                    finish_chunk(p, c, acc)
                if (p, c) == (0, 0):
                    wv_stack.close()
                if (p, c) == (0, NCH - 1):
                    kq_pool.release()
        attn_stack.close()


def build_program(masked=False, with_bias=False):
    key = (masked, with_bias)
    if key in _PROGRAM_CACHE:
        return _PROGRAM_CACHE[key]
    nc = bacc.Bacc("TRN2", target_bir_lowering=False, debug=False,
                   enable_asserts=False)
    xT = nc.dram_tensor("xT", [H, S], F32R, kind="ExternalInput").ap()
    wq = nc.dram_tensor("wq", [H, HD], F32R, kind="ExternalInput").ap()
    wk = nc.dram_tensor("wk", [H, HD], F32R, kind="ExternalInput").ap()
    wv = nc.dram_tensor("wv", [H, HD], F32R, kind="ExternalInput").ap()
    wo = nc.dram_tensor("wo", [HD, H], BF16, kind="ExternalInput").ap()
    bq = nc.dram_tensor("bq", [1, HD], F32R, kind="ExternalInput").ap()
    bk = nc.dram_tensor("bk", [1, HD], F32R, kind="ExternalInput").ap()
    bv = nc.dram_tensor("bv", [1, HD], F32R, kind="ExternalInput").ap()
    am = nc.dram_tensor("am", [P, KT_S], F32, kind="ExternalInput").ap()
    o = nc.dram_tensor("o_part", [S, H], BF16, kind="ExternalOutput").ap()
    with tile.TileContext(nc) as tc:
        _emit(tc, nc, (xT, wq, wk, wv, wo, bq, bk, bv, am, o), masked, with_bias)
    nc.compile()
    _PROGRAM_CACHE[key] = nc
    return nc


def _round_fp32r(a):
    """Round fp32 to the PE's fp32r format (11 mantissa bits, RNE)."""
    u = np.ascontiguousarray(a, np.float32).view(np.uint32)
    r = (u + np.uint32(0x7FF) + ((u >> np.uint32(12)) & np.uint32(1))) \
        & np.uint32(0xFFFFF000)
    return r.view(np.float32)


def _to_bf16(a):
    import ml_dtypes
    return np.ascontiguousarray(np.asarray(a, np.float32)).astype(
        ml_dtypes.bfloat16)


def make_in_maps(hidden_states, attention_mask, Wq, bq, Wk, bk, Wv, bv, Wo, bo):
    """Per-core input dicts. Core c: batch c//4, head-group c%4.

    Wq/bq are pre-scaled by 1/8 (= 1/sqrt(HEAD_DIM), exact in fp32) so the
    kernel's raw scores are already scaled. Tensors feeding float32r
    matmuls are pre-rounded to fp32r on the host; Wo is bf16.
    """
    hidden_states = np.asarray(hidden_states, np.float32)
    attention_mask = np.asarray(attention_mask, np.float32)
    xTs = [_round_fp32r(hidden_states[b].T) for b in range(B)]
    ams = []
    for b in range(B):
        amask = ((1.0 - attention_mask[b]) * -10000.0).astype(np.float32)
        ams.append(np.ascontiguousarray(amask.reshape(KT_S, P).T))
    in_maps = []
    for c in range(N_CORES):
        b, g = divmod(c, GROUPS)
        hs = slice(g * HD, (g + 1) * HD)
        in_maps.append({
            "xT": xTs[b],
            "wq": _round_fp32r(np.asarray(Wq, np.float32)[hs, :].T * np.float32(0.125)),
            "wk": _round_fp32r(np.asarray(Wk, np.float32)[hs, :].T),
            "wv": _round_fp32r(np.asarray(Wv, np.float32)[hs, :].T),
            "wo": _to_bf16(np.asarray(Wo, np.float32)[:, hs].T),
            "bq": _round_fp32r(np.asarray(bq, np.float32)[hs].reshape(1, HD) * np.float32(0.125)),
            "bk": _round_fp32r(np.asarray(bk, np.float32)[hs].reshape(1, HD)),
            "bv": _round_fp32r(np.asarray(bv, np.float32)[hs].reshape(1, HD)),
            "am": ams[b],
        })
    return in_maps


def kernel(hidden_states, attention_mask, Wq, bq, Wk, bk, Wv, bv, Wo, bo):
    masked = not bool(np.all(np.asarray(attention_mask) == 1.0))
    with_bias = not (np.all(np.asarray(bq) == 0) and np.all(np.asarray(bk) == 0)
                     and np.all(np.asarray(bv) == 0))
    nc = build_program(masked, with_bias)
    in_maps = make_in_maps(hidden_states, attention_mask,
                           Wq, bq, Wk, bk, Wv, bv, Wo, bo)
    res = run_bass_kernel_spmd(nc, in_maps, core_ids=list(range(N_CORES)))
    out = np.zeros((B, S, H), np.float32)
    for c in range(N_CORES):
        b = c // GROUPS
        out[b] += np.asarray(res.results[c]["o_part"], np.float32)
    out += np.asarray(bo, np.float32)
    return out


# revision 14
# speedup vs baseline: 1.3035x; 1.0896x over previous
"""Trainium2 Bass kernel: 16-head attention block (B=2, S=2048, H=1024).

Sharding: 8 cores = 2-way data parallel (batch) x 4-way tensor parallel
(head groups of 4 heads / 256 dims). Each core computes, for its batch
and head group:
    Q^T, K^T (= W @ x^T, [dims, seq] layout; Wq/bq pre-scaled by 1/8 on
    host so no score scaling is needed on device), V ([seq, dims]),
    S^T = K Q^T per head (key positions on partitions),
    P^T = exp(S^T + mask)  (bf16),
    ctx  = P^T.T @ [V | 1 | 0] per 128-query block (full 128-wide
           contraction; col 64 = softmax denominator, col 65 = pad),
    ctx normalized on DVE, DMA-transposed to ctx^T, then partial output
    O_g = ctx^T.T @ Wo[:,hs]^T.
Host sums the 4 partial outputs per batch and adds bo.

Projections and scores run in float32r (full-rate on the PE at free
size >= 256).  The probability matrix and V are bf16 so the flipped ctx
matmuls (N=66) keep LDWEIGHTS (fast-weight-load) under the 66-cycle
streams.  The whole kernel is one software-pipelined loop over 128
(pair, chunk, kt) slots: each slot emits the NEXT slot's score matmuls,
then a small PE "filler" piece (pair-1 projections / output projection),
then this slot's ctx matmuls, so the in-order PE queue never parks
behind an exp wait and the ACT engine (the 1.04us/slot bottleneck)
stays saturated.
"""

import contextlib
from collections import defaultdict

import numpy as np

import concourse.bass as bass
import concourse.mybir as mybir
import concourse.tile as tile
from concourse import bacc
from concourse.bass_utils import run_bass_kernel_spmd
from concourse.masks import make_identity

B, S, H = 2, 2048, 1024
NUM_HEADS, HEAD_DIM = 16, 64
N_CORES = 8
GROUPS = 4                  # head-parallel groups per batch
HD = H // GROUPS            # 256 head-dims per core (4 heads)
P = 128
KT_H = H // P               # 8 k-tiles over hidden dim
KT_S = S // P               # 16 k-tiles over sequence (key positions)
NCH = 4                     # q chunks per head pair
CHUNK = S // NCH            # 512
NQB = CHUNK // P            # 4 query blocks per chunk
NV = 66                     # V row: 64 dims + ones col + pad col
F32 = mybir.dt.float32
F32R = mybir.dt.float32r
BF16 = mybir.dt.bfloat16
EXP = mybir.ActivationFunctionType.Exp

_PROGRAM_CACHE = {}


def _emit(tc, nc, dram, masked, with_bias):
    mm = nc.tensor.matmul
    xT_d, wq_d, wk_d, wv_d, wo_d, bq_d, bk_d, bv_d, am_d, o_d = dram

    stack = contextlib.ExitStack()
    with stack:
        const = stack.enter_context(tc.tile_pool(name="const", bufs=1))
        big = stack.enter_context(tc.tile_pool(name="big", bufs=1))

        onesf = const.tile([P, 64], F32)
        nc.any.memset(onesf[:], 1.0)
        # warm the ACT exp table before it is first needed
        trash = const.tile([1, 16], F32)
        nc.scalar.activation(trash[:], onesf[0:1, 0:16], EXP)
        if masked:
            amask_sb = const.tile([P, KT_S], F32)
            nc.sync.dma_start(out=amask_sb[:], in_=am_d[:])
        bq_sb = bk_sb = bv_sb = ones_sb = None
        if with_bias:
            ones_sb = const.tile([1, 512], BF16)
            for i in range(8):
                nc.vector.tensor_copy(ones_sb[0:1, i * 64:(i + 1) * 64],
                                      onesf[0:1, :])
            bq_sb = const.tile([1, HD], BF16)
            nc.sync.dma_start(out=bq_sb[:], in_=bq_d[:])
            bk_sb = const.tile([1, HD], BF16)
            nc.sync.dma_start(out=bk_sb[:], in_=bk_d[:])
            bv_sb = const.tile([1, HD], BF16)
            nc.sync.dma_start(out=bv_sb[:], in_=bv_d[:])
        wo_sb = const.tile([P, HD // P, H], BF16)

        # persistent activations
        qT_sb = big.tile([P, 2, S], F32R)    # [dim-in-pair, pair, seq]
        kT_sb = big.tile([P, 2, S], F32R)
        v_sb = big.tile([P, KT_S, GROUPS, NV], BF16)  # [seq, kt, head, 66]
        ctxT_sb = big.tile([P, 2, S], BF16)

        warm_src = const.tile([P, 512], BF16)
        nc.any.memset(warm_src[:], 0.0)
        # ones column (softmax denominator) and zero pad column of V'
        nc.any.memset(v_sb[:, :, :, HEAD_DIM:HEAD_DIM + 1], 1.0)
        nc.any.memset(v_sb[:, :, :, HEAD_DIM + 1:NV], 0.0)
        # identity for the tail's PE transposes
        ident = const.tile([P, P], BF16)
        make_identity(nc, ident[:])

        # ---------- input DMAs: wk, wq, xT-cc0 gate the first exp; wv gates
        # chunk-0 ctx; later xT chunks stream under chunk-0's slots ----------
        kq_pool = tc.alloc_tile_pool(name="kq_pool", bufs=1, side="right")
        wv_stack = contextlib.ExitStack()
        wv_pool = wv_stack.enter_context(tc.tile_pool(name="wv_pool", bufs=1, side="right"))
        wk_sb = kq_pool.tile([P, KT_H, HD], BF16)
        xT_sb = kq_pool.tile([P, KT_H, S], BF16)
        wq_sb = kq_pool.tile([P, KT_H, HD], BF16)
        wv_sb = wv_pool.tile([P, KT_H, HD], BF16)
        xT_dr = xT_d.rearrange("(t p) s -> p t s", p=P)
        nc.sync.dma_start(out=wk_sb[:], in_=wk_d.rearrange("(t p) c -> p t c", p=P))
        nc.sync.dma_start(out=wq_sb[:], in_=wq_d.rearrange("(t p) c -> p t c", p=P))

        def xT_cc_dma(cc):
            for h in range(2):
                nc.sync.dma_start(
                    out=xT_sb[:, 4 * h:4 * h + 4, cc * 512:(cc + 1) * 512],
                    in_=xT_dr[:, 4 * h:4 * h + 4, cc * 512:(cc + 1) * 512])
        xT_cc_dma(0)
        nc.sync.dma_start(out=wv_sb[:], in_=wv_d.rearrange("(t p) c -> p t c", p=P))
        for cc in range(1, 4):
            xT_cc_dma(cc)
        nc.sync.dma_start(out=wo_sb[:], in_=wo_d.rearrange("(t p) c -> p t c", p=P))

        # ---------- PSUM pools: 4 banks scores (double-buffered), 2 banks
        # ctx accumulators, 2 banks misc (projections / output proj) ----------
        attn_stack = contextlib.ExitStack()
        sp_pool = attn_stack.enter_context(
            tc.tile_pool(name="sp_psum", bufs=2, space="PSUM"))
        acc_pool = attn_stack.enter_context(
            tc.tile_pool(name="acc_psum", bufs=1, space="PSUM"))
        misc_ps = attn_stack.enter_context(
            tc.tile_pool(name="misc_psum", bufs=2, space="PSUM"))
        ptp = attn_stack.enter_context(tc.tile_pool(name="pt_pool", bufs=6))
        npool = attn_stack.enter_context(tc.tile_pool(name="norm_pool", bufs=2))
        opool = attn_stack.enter_context(tc.tile_pool(name="o_pool", bufs=2))

        # PE p-state warm-up: burn the cold/mid HAM states on dummy matmuls
        # while the first input DMAs are still in flight.
        warm_ps = misc_ps.tile([P, 512], F32, tag="misc", name="warm_ps")
        for _ in range(6):
            mm(warm_ps[:], warm_src[:, 0:P], warm_src[:], start=True, stop=True)

        # ---- filler piece builders: each returned callable is <= ~0.9us PE ----
        def qk_proj_pieces(w_sb, b_sb, dst, p, cc, nsplit=2):
            """Split one K/Q projection group into `nsplit` pieces."""
            state = {}
            bounds = [KT_H * i // nsplit for i in range(nsplit + 1)]
            def piece(i):
                def run():
                    if i == 0:
                        state["ps"] = misc_ps.tile([P, 512], F32, tag="misc", name="ps_qk")
                    ps = state["ps"]
                    for kt in range(bounds[i], bounds[i + 1]):
                        mm(ps[:],
                           w_sb[:, kt, p * P:(p + 1) * P],
                           xT_sb[:, kt, cc * 512:(cc + 1) * 512],
                           start=(kt == 0),
                           stop=(not with_bias and kt == KT_H - 1))
                    if i == nsplit - 1:
                        if with_bias:
                            mm(ps[:], b_sb[:, p * P:(p + 1) * P],
                               ones_sb[:, 0:512], start=False, stop=True)
                        nc.vector.tensor_copy(
                            dst[:, p, cc * 512:(cc + 1) * 512], ps[:])
                return run
            return [piece(i) for i in range(nsplit)]

        def v_proj_piece(m, half):
            hs = slice(half * P, (half + 1) * P)
            def run():
                ps = misc_ps.tile([P, 512], F32, tag="misc", name="ps_v")
                for kt in range(KT_H):
                    mm(ps[:, 0:P],
                       xT_sb[:, kt, m * P:(m + 1) * P],
                       wv_sb[:, kt, hs],
                       start=(kt == 0), stop=(not with_bias and kt == KT_H - 1))
                if with_bias:
                    mm(ps[:, 0:P], ones_sb[:, 0:P], bv_sb[:, hs],
                       start=False, stop=True)
                nc.vector.tensor_copy(
                    v_sb[:, m, 2 * half:2 * half + 2, 0:HEAD_DIM],
                    ps[:, 0:P].rearrange("s (h d) -> s h d", d=HEAD_DIM))
            return run

        def oproj_pieces(m, on_act=False):
            state = {}
            def pe_piece(n2):
                def run():
                    ps = misc_ps.tile([P, 512], F32, tag="misc", name="ps_o")
                    state[n2] = ps
                    for kt2 in range(HD // P):
                        mm(ps[:],
                           ctxT_sb[:, kt2, m * P:(m + 1) * P],
                           wo_sb[:, kt2, n2 * 512:(n2 + 1) * 512],
                           start=(kt2 == 0), stop=(kt2 == HD // P - 1))
                return run
            def out_piece():
                o_sb = opool.tile([P, H], BF16, tag="o_sb", name="o_sb")
                for n2 in range(2):
                    if on_act:
                        nc.scalar.copy(o_sb[:, n2 * 512:(n2 + 1) * 512],
                                       state[n2][:])
                    else:
                        nc.vector.tensor_copy(
                            o_sb[:, n2 * 512:(n2 + 1) * 512], state[n2][:])
                nc.sync.dma_start(out=o_d[m * P:(m + 1) * P, :], in_=o_sb[:])
            return [pe_piece(0), pe_piece(1), out_piece]

        # ---------------- attention slot machinery ----------------
        def scores_kt(p, c, kt):
            """S^T tile for both heads of pair p: [128 keys, 2*512 q]."""
            sp = sp_pool.tile([P, 2, CHUNK], F32, tag="s_pair", name="sp")
            for hl in range(2):
                mm(sp[:, hl, :],
                   kT_sb[hl * 64:(hl + 1) * 64, p, kt * P:(kt + 1) * P],
                   qT_sb[hl * 64:(hl + 1) * 64, p, c * CHUNK:(c + 1) * CHUNK],
                   start=True, stop=True)
            pt = ptp.tile([P, 2, CHUNK], BF16, tag="pt", name="pt")
            if masked:
                nc.scalar.activation(pt[:], sp[:], EXP,
                                     bias=amask_sb[:, kt:kt + 1])
            else:
                nc.scalar.activation(pt[:], sp[:], EXP)
            return pt

        def ctx_kt(p, c, kt, acc, pt):
            """Accumulate ctx blocks [128 q, 66] for all (hl, qb)."""
            for hl in range(2):
                for qb in range(NQB):
                    mm(acc[:, hl, qb * NV:(qb + 1) * NV],
                       pt[:, hl, qb * P:(qb + 1) * P],
                       v_sb[:, kt, 2 * p + hl, :],
                       start=(kt == 0 and qb == 0),
                       stop=(kt == KT_S - 1 and qb == NQB - 1))

        def finish_chunk(p, c, acc):
            """Normalize ctx (DVE) and DMA-transpose into ctxT_sb."""
            rec = npool.tile([P, 2, NQB, 1], F32, tag="rec", name="rec")
            ctx_n = npool.tile([P, NQB, 2, HEAD_DIM], BF16, tag="ctx_n", name="ctx_n")
            for hl in range(2):
                blk = acc[:, hl, 0:NQB * NV].rearrange(
                    "q (qb c) -> q qb c", c=NV)
                nc.vector.reciprocal(rec[:, hl], blk[:, :, 64:65])
                nc.vector.tensor_mul(
                    ctx_n[:, :, hl, :], blk[:, :, 0:HEAD_DIM],
                    rec[:, hl].broadcast_to((P, NQB, HEAD_DIM)))
            for qb in range(NQB):
                nc.sync.dma_start_transpose(
                    out=ctxT_sb[:, p, c * CHUNK + qb * P:c * CHUNK + (qb + 1) * P],
                    in_=ctx_n[:, qb])

        def finish_chunk_tail(p, c, acc):
            """Last chunk: per-qblock normalize + PE transpose + output
            projection, pipelined so the tail is as short as possible."""
            rec = npool.tile([P, 2, NQB, 1], F32, tag="rec", name="rec")
            ctx_n = npool.tile([P, NQB, 2, HEAD_DIM], BF16, tag="ctx_n", name="ctx_n")
            blks = [acc[:, hl, 0:NQB * NV].rearrange("q (qb c) -> q qb c", c=NV)
                    for hl in range(2)]
            for qb in range(NQB):
                for hl in range(2):
                    nc.vector.reciprocal(rec[:, hl, qb], blks[hl][:, qb, 64:65])
                    nc.vector.tensor_mul(
                        ctx_n[:, qb, hl, :], blks[hl][:, qb, 0:HEAD_DIM],
                        rec[:, hl, qb].broadcast_to((P, HEAD_DIM)))
                tr_ps = misc_ps.tile([P, 512], F32, tag="misc", name="tr_ps")
                tr_bf = tr_ps[:].bitcast(BF16)
                nc.tensor.transpose(tr_bf[:, 0:P], ctx_n[:, qb], ident[:])
                dst = ctxT_sb[:, p, c * CHUNK + qb * P:c * CHUNK + (qb + 1) * P]
                nc.vector.tensor_copy(dst, tr_bf[:, 0:P])
                m = c * NQB + qb
                for pc in oproj_pieces(m, on_act=True):
                    pc()

        # ---------------- piece schedule over the 128 slots ----------------
        # slot index = (p*NCH + c)*KT_S + kt.  Deadline-driven: each filler
        # is placed as late as its consumer allows, spreading the PE load so
        # no slot prefix exceeds the ACT-paced budget.
        sched = defaultdict(list)

        def place(slots_list, pieces):
            for s, pc in zip(slots_list, pieces):
                sched[s].append(pc)

        def kq(w, b, dst, p, cc, n=2):
            return qk_proj_pieces(w, b if with_bias else None, dst, p, cc,
                                  nsplit=n)

        # pair-0 V (heads 0/1): needed by ctx(p0,c0,kt=m); pieces run after
        # ctx in each slot, so place in slot m-1 (V-A(0) goes pre-loop)
        for m in range(1, KT_S):
            sched[m - 1].append(v_proj_piece(m, 0))
        # pair-1 V (heads 2/3): needed from slot 64; spread over chunks 1-2
        for m in range(KT_S):
            sched[16 + 2 * m].append(v_proj_piece(m, 1))
        # pair-0 K: K(cc) gates scores(kt=4cc) emitted at slot 4cc-1
        place([0, 1], kq(wk_sb, bk_sb, kT_sb, 0, 1))
        place([4, 5], kq(wk_sb, bk_sb, kT_sb, 0, 2))
        place([8, 9], kq(wk_sb, bk_sb, kT_sb, 0, 3))
        # pair-0 Q: Q(cc) gates chunk cc's scores (slot 16cc-1)
        place([12, 13], kq(wq_sb, bq_sb, qT_sb, 0, 1))
        place([20, 21], kq(wq_sb, bq_sb, qT_sb, 0, 2))
        place([36, 37], kq(wq_sb, bq_sb, qT_sb, 0, 3))
        # pair-1 K/Q: cc0 gates slot 63's scores; later ccs gate pair-1 slots
        place([48, 50], kq(wk_sb, bk_sb, kT_sb, 1, 0))
        place([52, 54], kq(wq_sb, bq_sb, qT_sb, 1, 0))
        place([56, 58], kq(wk_sb, bk_sb, kT_sb, 1, 1))
        place([60, 62], kq(wk_sb, bk_sb, kT_sb, 1, 2))
        place([64, 66], kq(wk_sb, bk_sb, kT_sb, 1, 3))
        place([70, 72], kq(wq_sb, bq_sb, qT_sb, 1, 1))
        place([76, 78], kq(wq_sb, bq_sb, qT_sb, 1, 2))
        place([86, 88], kq(wq_sb, bq_sb, qT_sb, 1, 3))
        # output projection for chunk c: slots of pair-1 chunk c+1
        for c in range(NCH - 1):
            for m in range(4 * c, 4 * c + 4):
                base = (NCH + c + 1) * KT_S
                for i, pc in enumerate(oproj_pieces(m)):
                    sched[base + (m % 4) * 4 + i].append(pc)

        # ---------------- phase 1: pair-0 cc0 K/Q projection ----------------
        ps_k = misc_ps.tile([P, 512], F32, tag="misc")
        for kt in range(KT_H):
            mm(ps_k[:], wk_sb[:, kt, 0:P], xT_sb[:, kt, 0:512],
               start=(kt == 0), stop=(not with_bias and kt == KT_H - 1))
        if with_bias:
            mm(ps_k[:], bk_sb[:, 0:P], ones_sb[:, 0:512], start=False, stop=True)
        nc.vector.tensor_copy(kT_sb[:, 0, 0:512], ps_k[:])
        ps_q = misc_ps.tile([P, 512], F32, tag="misc")
        for kt in range(KT_H):
            mm(ps_q[:], wq_sb[:, kt, 0:P], xT_sb[:, kt, 0:512],
               start=(kt == 0), stop=(not with_bias and kt == KT_H - 1))
        if with_bias:
            mm(ps_q[:], bq_sb[:, 0:P], ones_sb[:, 0:512], start=False, stop=True)
        nc.vector.tensor_copy(qT_sb[:, 0, 0:512], ps_q[:])

        # ---------------- phase 2: the software-pipelined slot loop ----------
        slots = [(p, c, kt)
                 for p in range(2) for c in range(NCH) for kt in range(KT_S)]
        acc = None
        pt_next = scores_kt(0, 0, 0)
        v_proj_piece(0, 0)()
        for i, (p, c, kt) in enumerate(slots):
            pt_cur = pt_next
            if i + 1 < len(slots):
                pn, cn, ktn = slots[i + 1]
                pt_next = scores_kt(pn, cn, ktn)
            if kt == 0:
                acc = acc_pool.tile([P, 2, 512], F32, tag="acc", name="acc")
            ctx_kt(p, c, kt, acc, pt_cur)
            for piece in sched.get(i, ()):
                piece()
            if kt == KT_S - 1:
                if (p, c) == (1, NCH - 1):
                    finish_chunk_tail(p, c, acc)
                else:
                    finish_chunk(p, c, acc)
                if (p, c) == (0, NCH - 1):
                    wv_stack.close()
                if (p, c) == (1, 2):
                    kq_pool.release()
        attn_stack.close()


def build_program(masked=False, with_bias=False):
    key = (masked, with_bias)
    if key in _PROGRAM_CACHE:
        return _PROGRAM_CACHE[key]
    nc = bacc.Bacc("TRN2", target_bir_lowering=False, debug=False,
                   enable_asserts=False)
    xT = nc.dram_tensor("xT", [H, S], BF16, kind="ExternalInput").ap()
    wq = nc.dram_tensor("wq", [H, HD], BF16, kind="ExternalInput").ap()
    wk = nc.dram_tensor("wk", [H, HD], BF16, kind="ExternalInput").ap()
    wv = nc.dram_tensor("wv", [H, HD], BF16, kind="ExternalInput").ap()
    wo = nc.dram_tensor("wo", [HD, H], BF16, kind="ExternalInput").ap()
    bq = nc.dram_tensor("bq", [1, HD], BF16, kind="ExternalInput").ap()
    bk = nc.dram_tensor("bk", [1, HD], BF16, kind="ExternalInput").ap()
    bv = nc.dram_tensor("bv", [1, HD], BF16, kind="ExternalInput").ap()
    am = nc.dram_tensor("am", [P, KT_S], F32, kind="ExternalInput").ap()
    o = nc.dram_tensor("o_part", [S, H], BF16, kind="ExternalOutput").ap()
    with tile.TileContext(nc) as tc:
        _emit(tc, nc, (xT, wq, wk, wv, wo, bq, bk, bv, am, o), masked, with_bias)
    nc.compile()
    _PROGRAM_CACHE[key] = nc
    return nc


def _round_fp32r(a):
    """Round fp32 to the PE's fp32r format (11 mantissa bits, RNE)."""
    u = np.ascontiguousarray(a, np.float32).view(np.uint32)
    r = (u + np.uint32(0x7FF) + ((u >> np.uint32(12)) & np.uint32(1))) \
        & np.uint32(0xFFFFF000)
    return r.view(np.float32)


def _to_bf16(a):
    import ml_dtypes
    return np.ascontiguousarray(np.asarray(a, np.float32)).astype(
        ml_dtypes.bfloat16)


def make_in_maps(hidden_states, attention_mask, Wq, bq, Wk, bk, Wv, bv, Wo, bo):
    """Per-core input dicts. Core c: batch c//4, head-group c%4.

    Wq/bq are pre-scaled by 1/8 (= 1/sqrt(HEAD_DIM), exact in fp32) so the
    kernel's raw scores are already scaled. Tensors feeding float32r
    matmuls are pre-rounded to fp32r on the host; Wo is bf16.
    """
    hidden_states = np.asarray(hidden_states, np.float32)
    attention_mask = np.asarray(attention_mask, np.float32)
    xTs = [_to_bf16(hidden_states[b].T) for b in range(B)]
    ams = []
    for b in range(B):
        amask = ((1.0 - attention_mask[b]) * -10000.0).astype(np.float32)
        ams.append(np.ascontiguousarray(amask.reshape(KT_S, P).T))
    in_maps = []
    for c in range(N_CORES):
        b, g = divmod(c, GROUPS)
        hs = slice(g * HD, (g + 1) * HD)
        in_maps.append({
            "xT": xTs[b],
            "wq": _to_bf16(np.asarray(Wq, np.float32)[hs, :].T * np.float32(0.125)),
            "wk": _to_bf16(np.asarray(Wk, np.float32)[hs, :].T),
            "wv": _to_bf16(np.asarray(Wv, np.float32)[hs, :].T),
            "wo": _to_bf16(np.asarray(Wo, np.float32)[:, hs].T),
            "bq": _to_bf16(np.asarray(bq, np.float32)[hs].reshape(1, HD) * np.float32(0.125)),
            "bk": _to_bf16(np.asarray(bk, np.float32)[hs].reshape(1, HD)),
            "bv": _to_bf16(np.asarray(bv, np.float32)[hs].reshape(1, HD)),
            "am": ams[b],
        })
    return in_maps


def kernel(hidden_states, attention_mask, Wq, bq, Wk, bk, Wv, bv, Wo, bo):
    masked = not bool(np.all(np.asarray(attention_mask) == 1.0))
    with_bias = not (np.all(np.asarray(bq) == 0) and np.all(np.asarray(bk) == 0)
                     and np.all(np.asarray(bv) == 0))
    nc = build_program(masked, with_bias)
    in_maps = make_in_maps(hidden_states, attention_mask,
                           Wq, bq, Wk, bk, Wv, bv, Wo, bo)
    res = run_bass_kernel_spmd(nc, in_maps, core_ids=list(range(N_CORES)))
    out = np.zeros((B, S, H), np.float32)
    for c in range(N_CORES):
        b = c // GROUPS
        out[b] += np.asarray(res.results[c]["o_part"], np.float32)
    out += np.asarray(bo, np.float32)
    return out
